# revision 1
# baseline (speedup 1.0000x reference)
"""Trainium2 Bass kernel for nn_FLASH_ShareA_FFConvM.

Strategy: data-parallel over (batch, seq-half): 8 cores, each handling 4096
tokens (16 local-attention chunks of 256). All weights replicated. Per core:

  phase A (token-major): LayerNorm stats + normalized x, bf16 copy,
    DMA-transpose into channel-major xs^T [512c x 4224t] (col j = token j-128;
    128-row halo tile at the front provides the token-shift source).
    The "shift first half of channels by one token" becomes a column-offset
    AP on xs^T c-chunks 0..1.
  phase B (per chunk g): qk^T = wqk^T @ xs_sh^T; per-chunk mean stats ->
    OffsetScale -> q/k/qs/ks (ACT per-partition scale+bias, shift = col AP);
    sim^T[j,i] per head via PE (K=64); mask+exp; softmax denominator via
    ones-matmul; reciprocal broadcast via DRAM bounce; attn = exp * rec;
    V-matmul out^T[e,(h,i)] with token-major hidden as stationary;
    silu*gate -> og^T; fin[t,d] PSUM accumulates xs_sh@W_comb (folded
    hidden@w_out[:512]) + og@w_out[512:]; finalize y = fin*silu(vgate) + xs_sh
    with fp32 xs recomputed from saved LN stats (token-shift via SBUF DMA).
"""

import sys

sys.path.insert(0, "/opt/trn_rl_repo")

import numpy as np
import ml_dtypes
from contextlib import ExitStack

import concourse.bass as bass
import concourse.tile as tile
from concourse import bacc, mybir

F32 = mybir.dt.float32
BF16 = mybir.dt.bfloat16
AX = mybir.AxisListType
ALU = mybir.AluOpType
ACTF = mybir.ActivationFunctionType

B, SEQ, DIM = 4, 8192, 512
G, QK = 32, 128
CHUNK = SEQ // G          # 256 tokens per attention chunk
HD = QK // 4              # 32 (softmax scale dim, per source)
SCALE = float(HD) ** -0.5
HID = DIM                 # 512
EPS = 1e-5
N_CORES = 8
T_CORE = SEQ // 2         # 4096 tokens per core
NEG = -1.0e30

BF = ml_dtypes.bfloat16


def build_core_program(ctx: ExitStack, tc, aps, n_tok, apply_g, apply_b,
                       pe_transpose=False, silu_native=True):
    """Emit the per-core program. aps: dict name -> bass.AP (DRAM)."""
    nc = tc.nc
    n_tiles = n_tok // 128            # 128-token tiles (excl. halo tile)
    n_chunks = n_tok // CHUNK
    n_pad = n_tok + 128               # halo tile rows 0..127 (token j-128)

    xp = aps["xp"]; yout = aps["y"]

    # ---------------- pools ----------------
    consts = ctx.enter_context(tc.tile_pool(name="consts", bufs=1))
    persist = ctx.enter_context(tc.tile_pool(name="persist", bufs=1))
    work = ctx.enter_context(tc.tile_pool(name="work", bufs=1))
    psum = ctx.enter_context(tc.tile_pool(name="psum", bufs=1, space="PSUM"))
    dram = ctx.enter_context(tc.tile_pool(name="dram", bufs=1, space="DRAM"))

    # ---------------- constants into SBUF ----------------
    def cload(name, shape, dtype):
        """Load a [R, C] DRAM const into SBUF; R>128 folds to [128, R//128, C]
        (row r = cc*128 + p -> tile[p, cc, :]), returning slices via [:, cc, :].
        """
        if shape[0] > 128:
            k = shape[0] // 128
            t = consts.tile([128, k, shape[1]], dtype, name=f"c_{name}",
                            tag=f"c_{name}")
            nc.sync.dma_start(t[:], aps[name].rearrange("(k p) c -> p k c",
                                                        p=128))
        else:
            t = consts.tile(shape, dtype, name=f"c_{name}", tag=f"c_{name}")
            nc.sync.dma_start(t[:], aps[name])
        return t

    wqk = cload("wqk", [512, 128], BF16)      # [128c, 4cc, 128d]
    whvg = cload("whvg", [512, 1024], BF16)   # moving: cols 0:512 hid, 512: vgate
    wga = cload("wga", [512, 512], BF16)      # attn gate; lhsT tiles
    wcomb = cload("wcomb", [512, 512], BF16)  # moving rhs for fin part1
    woa = cload("woa", [2048, 512], BF16)     # moving rhs for fin part2
    g46 = cload("g46", [128, 6], F32)         # gammas.T / CHUNK (4 + 2)
    b46 = cload("b46", [128, 6], F32)         # betas.T
    maskt = cload("maskt", [256, 256], F32)   # [j, i]: 0 if j<=i else NEG
    if apply_g:
        lng = cload("lng", [128, 512], F32)
    if apply_b:
        lnb = cload("lnb", [128, 512], F32)
    identb = cload("identb", [128, 128], BF16)

    ones_bf = consts.tile([128, 1], BF16, name="ones_bf", tag="ones_bf")
    nc.vector.memset(ones_bf[:], 1.0)
    epsb = consts.tile([128, 1], F32, name="epsb", tag="epsb")
    nc.vector.memset(epsb[:], EPS)

    # ---------------- persistent state ----------------
    # channel-major normalized+g-scaled x, bf16; col j = token (j - 128)
    # folded: xsT[p, cc, j] = xs[token j-128, channel cc*128+p]
    xsT = persist.tile([128, 4, n_pad], BF16, name="xsT", tag="xsT")
    # per-LN-tile stats: cols 2i (mean), 2i+1 (rstd); tile i covers xp rows
    # [i*128, (i+1)*128)
    stats = persist.tile([128, 2 * (n_tiles + 1)], F32, name="stats",
                         tag="stats")

    def act(bi):
        """Chain ACT-table ops into a fixed order to avoid table thrash."""
        tc.chain_iter_dep("actfn", bi.ins)
        return bi

    # ---------------- phase A1: LN stats for all tiles (ACT-chain head) ---
    def emit_stats_tile(i):
        x_t = work.tile([128, 512], F32, name=f"xa{i}", tag="xa", bufs=4)
        nc.sync.dma_start(x_t[:], xp[i * 128:(i + 1) * 128, :])
        bns = work.tile([128, 6], F32, name=f"bns{i}", tag="bns", bufs=2)
        nc.vector.bn_stats(out=bns[:], in_=x_t[:])
        mv = work.tile([128, 2], F32, name=f"mv{i}", tag="mv", bufs=2)
        nc.vector.bn_aggr(out=mv[:], in_=bns[:])
        nc.vector.tensor_copy(out=stats[:, 2 * i:2 * i + 1], in_=mv[:, 0:1])
        # rstd = 1/sqrt(var+eps)
        sd = work.tile([128, 1], F32, name=f"sd{i}", tag="sd", bufs=2)
        act(nc.scalar.activation(out=sd[:], in_=mv[:, 1:2], func=ACTF.Sqrt,
                                 bias=epsb[:]))
        nc.vector.reciprocal(out=sd[:], in_=sd[:])
        nc.vector.tensor_copy(out=stats[:, 2 * i + 1:2 * i + 2], in_=sd[:])

    # ---------------- phase A2: normalized bf16 x + transpose (JIT) -------
    def emit_ln_tile(i):
        xs_bf = work.tile([128, 512], BF16, name=f"xsbf{i}", tag="xsbf",
                          bufs=3)
        xs_src = emit_xs_norm(i, xs_bf)
        # transpose [128t, 512c] -> xsT[:, :, i*128:(i+1)*128] in one DMA
        nc.sync.dma_start(xsT[:, :, i * 128:(i + 1) * 128], xs_bf[:],
                          transpose=True)

    def emit_xs_norm(i, out_tile):
        x_t = work.tile([128, 512], F32, name=f"xn{i}", tag="xn", bufs=3)
        nc.sync.dma_start(x_t[:], xp[i * 128:(i + 1) * 128, :])
        mean = stats[:, 2 * i:2 * i + 1]
        rstd = stats[:, 2 * i + 1:2 * i + 2]
        if apply_g or apply_b:
            xs_f = work.tile([128, 512], F32, name=f"xsf{i}", tag="xsf",
                             bufs=2)
            nc.vector.tensor_scalar(out=xs_f[:], in0=x_t[:],
                                    scalar1=mean, scalar2=rstd,
                                    op0=ALU.subtract, op1=ALU.mult)
            if apply_g and apply_b:
                nc.vector.tensor_mul(out=xs_f[:], in0=xs_f[:], in1=lng[:])
                nc.vector.tensor_add(out=out_tile[:], in0=xs_f[:],
                                     in1=lnb[:])
            elif apply_g:
                nc.vector.tensor_mul(out=out_tile[:], in0=xs_f[:],
                                     in1=lng[:])
            else:
                nc.vector.tensor_add(out=out_tile[:], in0=xs_f[:],
                                     in1=lnb[:])
        else:
            nc.vector.tensor_scalar(out=out_tile[:], in0=x_t[:],
                                    scalar1=mean, scalar2=rstd,
                                    op0=ALU.subtract, op1=ALU.mult)

    # ---------------- halo xs (token-major, fp32) for residual shift -------
    def emit_xs_tok(ti):
        """Recompute fp32 token-major xs for xp tile ti (rows ti*128..)."""
        xs = work.tile([128, 512], F32, name=f"xstok{ti}", tag="xstok",
                       bufs=3)
        emit_xs_norm(ti, xs)
        return xs

    # interleave: LN tiles stay a couple ahead of the chunk that needs them
    ln_next = 0

    def ensure_ln(upto):
        nonlocal ln_next
        while ln_next <= min(upto, n_tiles):
            emit_ln_tile(ln_next)
            ln_next += 1

    # ---------------- phase B: chunk pairs ----------------
    stats_next = 0

    def ensure_stats(upto):
        nonlocal stats_next
        while stats_next <= min(upto, n_tiles):
            emit_stats_tile(stats_next)
            stats_next += 1

    ensure_stats(8)
    ensure_ln(2)
    prev_xs = emit_xs_tok(0)   # halo tile; only row 127 is ever read

    def xsh(cc, col0, width):
        c0 = col0 - 1 if cc < 2 else col0
        return xsT[:, cc, c0:c0 + width]

    pending = []
    for gp in range(n_chunks // 2):
        ensure_stats(4 * gp + 12)
        ensure_ln(4 * gp + 6)
        # (stats for all tiles already emitted; this emits bf16+transpose)
        colP = 128 + gp * 2 * CHUNK      # pair start col (unshifted)

        # --- qk^T for the pair [128qk, 512] ---
        qkps = psum.tile([128, 512], F32, name=f"qkps{gp}", tag="mm_s",
                         bufs=3)
        for cc in range(4):
            nc.tensor.matmul(qkps[:], wqk[:, cc, :], xsh(cc, colP, 512),
                             start=(cc == 0), stop=(cc == 3))
        qkT = work.tile([128, 512], F32, name=f"qkT{gp}", tag="qkT", bufs=2)
        nc.scalar.copy(out=qkT[:], in_=qkps[:])

        # --- attn gate^T for the pair (channel-major) ---
        gate_bf = []
        for ee in range(4):
            gps = psum.tile([128, 512], F32, name=f"g{gp}_{ee}", tag="mm_s",
                            bufs=3)
            for cc in range(4):
                nc.tensor.matmul(gps[:], wga[:, cc, ee * 128:(ee + 1) * 128],
                                 xsh(cc, colP, 512),
                                 start=(cc == 0), stop=(cc == 3))
            gb = work.tile([128, 512], BF16, name=f"gate{gp}_{ee}",
                           tag=f"gate{ee}", bufs=2)
            nc.scalar.copy(out=gb[:], in_=gps[:])
            gate_bf.append(gb)

        for g in (2 * gp, 2 * gp + 1):
            half = g % 2
            colU = 128 + g * CHUNK
            qk_c = qkT[:, half * 256:(half + 1) * 256]

            # --- per-chunk stats -> offsets/scales [128,1] each ---
            qsum = work.tile([128, 1], F32, name=f"qsum{g}", tag="qsum",
                             bufs=2)
            nc.vector.tensor_reduce(out=qsum[:], in_=qk_c, axis=AX.X,
                                    op=ALU.add)
            offs = work.tile([128, 6], F32, name=f"offs{g}", tag="offs",
                             bufs=2)
            # offs[:, i] = qk_sum * gamma_i/CHUNK + beta_i  (one DVE op)
            nc.vector.scalar_tensor_tensor(out=offs[:], in0=g46[:],
                                           scalar=qsum[:], in1=b46[:],
                                           op0=ALU.mult, op1=ALU.add)
            qoff, koff, qsc, ksc, qsoff, ksoff = (offs[:, i:i + 1]
                                                  for i in range(6))

            # --- q/k/qs/ks channel-major bf16 [128qk, 256] ---
            qT = work.tile([128, 256], BF16, name=f"qT{g}", tag="qT", bufs=2)
            kT = work.tile([128, 256], BF16, name=f"kT{g}", tag="kT", bufs=2)
            nc.vector.tensor_scalar(out=qT[:], in0=qk_c, scalar1=qsc,
                                    scalar2=qoff, op0=ALU.mult, op1=ALU.add)
            nc.vector.tensor_scalar(out=kT[:], in0=qk_c, scalar1=ksc,
                                    scalar2=koff, op0=ALU.mult, op1=ALU.add)
            qsT = work.tile([128, 256], BF16, name=f"qsT{g}", tag="qsT",
                            bufs=2)
            ksT = work.tile([128, 256], BF16, name=f"ksT{g}", tag="ksT",
                            bufs=2)
            nc.vector.tensor_copy(out=qsT[:, 0:1], in_=qsoff)
            nc.vector.tensor_copy(out=ksT[:, 0:1], in_=ksoff)
            nc.vector.tensor_scalar(out=qsT[:, 1:256], in0=qT[:, 0:255],
                                    scalar1=qsc, scalar2=qsoff, op0=ALU.mult,
                                    op1=ALU.add)
            nc.vector.tensor_scalar(out=ksT[:, 1:256], in0=kT[:, 0:255],
                                    scalar1=ksc, scalar2=ksoff, op0=ALU.mult,
                                    op1=ALU.add)

            # --- sim^T + mask + exp -> exp[jt] [128j, 4h*256i] bf16 ---
            expt = [work.tile([128, 1024], BF16, name=f"exp{g}_{jt}",
                              tag=f"exp{jt}", bufs=2) for jt in range(2)]
            for h in range(4):
                Q = qT if h < 2 else qsT
                K = kT if h < 2 else ksT
                dr = (h % 2) * 64
                for jt in range(2):
                    sim = psum.tile([128, 256], F32, name=f"sim{g}_{h}_{jt}",
                                    tag="mm_s", bufs=3)
                    nc.tensor.matmul(sim[:],
                                     K[dr:dr + 64, jt * 128:(jt + 1) * 128],
                                     Q[dr:dr + 64, :], start=True, stop=True)
                    tmp = work.tile([128, 256], F32, name=f"ms{g}_{h}_{jt}",
                                    tag="ms", bufs=3)
                    nc.vector.scalar_tensor_tensor(
                        out=tmp[:], in0=sim[:], scalar=SCALE,
                        in1=maskt[:, jt, :],
                        op0=ALU.mult, op1=ALU.add)
                    act(nc.scalar.activation(
                        out=expt[jt][:, h * 256:(h + 1) * 256],
                        in_=tmp[:], func=ACTF.Exp))

            # --- denominator + reciprocal + broadcast + attn ---
            sums = [psum.tile([1, 512], F32, name=f"sums{g}_{s}", tag="mm_s",
                              bufs=3) for s in range(2)]
            for jt in range(2):
                for s in range(2):
                    nc.tensor.matmul(sums[s][0:1, :],
                                     ones_bf[:],
                                     expt[jt][:, s * 512:(s + 1) * 512],
                                     start=(jt == 0), stop=(jt == 1))
            rec = work.tile([1, 1024], F32, name=f"rec{g}", tag="rec",
                            bufs=2)
            for s in range(2):
                nc.vector.reciprocal_approx_fast(
                    out=rec[:, s * 512:(s + 1) * 512], in_=sums[s][0:1, :])
            recd = dram.tile([1, 1024], F32, name=f"recd{g}", tag="recd",
                             bufs=2)
            nc.sync.dma_start(recd[:], rec[:])
            recb = work.tile([128, 1024], F32, name=f"recb{g}", tag="recb",
                             bufs=2)
            bcast = bass.AP(tensor=recd.tensor, offset=recd.offset,
                            ap=[[0, 128], list(recd.ap)[-1]])
            nc.sync.dma_start(recb[:], bcast)
            attn = [work.tile([128, 1024], BF16, name=f"attn{g}_{jt}",
                              tag=f"attn{jt}", bufs=2) for jt in range(2)]
            for jt in range(2):
                nc.vector.tensor_mul(out=attn[jt][:], in0=expt[jt][:],
                                     in1=recb[:])

            # --- hidden + vgate (token-major) for the chunk's 2 t-tiles ---
            hid_bf = []
            svg_bf = []
            for tt in range(2):
                ti = g * 2 + tt
                colT = 128 + ti * 128
                hvh = psum.tile([128, 512], F32, name=f"hvh{g}_{tt}",
                                tag="hvv", bufs=4)
                hvv2 = psum.tile([128, 512], F32, name=f"hvg{g}_{tt}",
                                 tag="hvv", bufs=4)
                for cc in range(4):
                    nc.tensor.matmul(hvh[:], xsh(cc, colT, 128),
                                     whvg[:, cc, 0:512],
                                     start=(cc == 0), stop=(cc == 3))
                for cc in range(4):
                    nc.tensor.matmul(hvv2[:], xsh(cc, colT, 128),
                                     whvg[:, cc, 512:1024],
                                     start=(cc == 0), stop=(cc == 3))
                hb = work.tile([128, 512], BF16, name=f"hid{g}_{tt}",
                               tag="hid", bufs=4)
                nc.scalar.copy(out=hb[:], in_=hvh[:])
                sv = work.tile([128, 512], BF16, name=f"svg{g}_{tt}",
                               tag="svg", bufs=4)
                if silu_native:
                    act(nc.scalar.activation(out=sv[:], in_=hvv2[:],
                                             func=ACTF.Silu))
                else:
                    sgt = work.tile([128, 512], F32, name=f"sg{g}_{tt}",
                                    tag="sgt", bufs=2)
                    act(nc.scalar.activation(out=sgt[:], in_=hvv2[:],
                                             func=ACTF.Sigmoid))
                    nc.vector.tensor_mul(out=sv[:], in0=sgt[:],
                                         in1=hvv2[:])
                hid_bf.append(hb)
                svg_bf.append(sv)


            def back_half(g=g, half=half, attn=attn, hid_bf=hid_bf,
                          svg_bf=svg_bf, gate_bf=gate_bf):
                nonlocal prev_xs
                # --- V matmul + silu + gate -> og^T [128e, 4h*256i] bf16 ---
                og_bf = []
                for ee in range(4):
                    osl = work.tile([128, 1024], BF16, name=f"osl{g}_{ee}",
                                    tag="osl", bufs=2)
                    for s in range(2):
                        vps = psum.tile([128, 512], F32,
                                        name=f"v{g}_{ee}_{s}",
                                        tag="hvv", bufs=4)
                        for jt in range(2):
                            nc.tensor.matmul(
                                vps[:],
                                hid_bf[jt][:, ee * 128:(ee + 1) * 128],
                                attn[jt][:, s * 512:(s + 1) * 512],
                                start=(jt == 0), stop=(jt == 1))
                        if silu_native:
                            act(nc.scalar.activation(
                                out=osl[:, s * 512:(s + 1) * 512],
                                in_=vps[:], func=ACTF.Silu))
                        else:
                            sgo = work.tile([128, 512], F32,
                                            name=f"sgo{g}_{ee}_{s}",
                                            tag="sgo", bufs=2)
                            act(nc.scalar.activation(out=sgo[:], in_=vps[:],
                                                     func=ACTF.Sigmoid))
                            nc.vector.tensor_mul(
                                out=osl[:, s * 512:(s + 1) * 512],
                                in0=sgo[:], in1=vps[:])
                    ob = work.tile([128, 1024], BF16, name=f"og{g}_{ee}",
                                   tag=f"og{ee}", bufs=2)
                    gslice = gate_bf[ee][:, half * 256:(half + 1) * 256]
                    gbc = gslice.unsqueeze(1).broadcast_to((128, 4, 256))
                    nc.vector.tensor_mul(
                        out=ob.rearrange("p (h i) -> p h i", h=4),
                        in0=osl.rearrange("p (h i) -> p h i", h=4),
                        in1=gbc)
                    og_bf.append(ob)

                # --- fin PSUM accumulation, then finalize each t-tile ---
                for tt in range(2):
                    ti = g * 2 + tt
                    colT = 128 + ti * 128
                    fin = psum.tile([128, 512], F32, name=f"fin{g}_{tt}",
                                    tag="fin", bufs=1)
                    for cc in range(4):
                        nc.tensor.matmul(fin[:], xsh(cc, colT, 128),
                                         wcomb[:, cc, :],
                                         start=(cc == 0), stop=False)
                    for h in range(4):
                        for ee in range(4):
                            ff = h * 4 + ee
                            nc.tensor.matmul(
                                fin[:],
                                og_bf[ee][:, h * 256 + tt * 128:
                                          h * 256 + tt * 128 + 128],
                                woa[:, ff, :],
                                start=False, stop=(ff == 15))

                    xs_cur = emit_xs_tok(ti + 1)
                    xsprev = work.tile([128, 256], F32, name=f"xsp{ti}",
                                       tag="xsp", bufs=2)
                    nc.sync.dma_start(xsprev[1:128, :], xs_cur[0:127, 0:256])
                    nc.sync.dma_start(xsprev[0:1, :], prev_xs[127:128, 0:256])
                    prev_xs = xs_cur

                    y = work.tile([128, 512], F32, name=f"y{ti}", tag="y",
                                  bufs=3)
                    nc.vector.tensor_mul(out=y[:], in0=fin[:],
                                         in1=svg_bf[tt][:])
                    nc.vector.tensor_add(out=y[:, 256:512], in0=y[:, 256:512],
                                         in1=xs_cur[:, 256:512])
                    nc.vector.tensor_add(out=y[:, 0:256], in0=y[:, 0:256],
                                         in1=xsprev[:])
                    nc.sync.dma_start(yout[ti * 128:(ti + 1) * 128, :], y[:])



            pending.append(back_half)
            if len(pending) > 1:
                pending.pop(0)()

    while pending:
        pending.pop(0)()

def make_host_inputs(x, ln_g, ln_b, w_qk, g4, b4, g2, b2, w_hidden, w_gate,
                     w_out, n_tok=T_CORE):
    """Build shared weight arrays + per-core xp slices."""
    x = np.asarray(x, np.float32)
    ln_g = np.asarray(ln_g, np.float32)
    ln_b = np.asarray(ln_b, np.float32)
    apply_g = not np.all(ln_g == 1.0)
    apply_b = bool(np.any(ln_b != 0.0))

    w_hidden = np.asarray(w_hidden, np.float32)
    w_out = np.asarray(w_out, np.float32)
    w_gate = np.asarray(w_gate, np.float32)
    w_qk = np.asarray(w_qk, np.float32)

    wcomb = (w_hidden[:, :HID] @ w_out[:HID, :]).astype(np.float32)

    jj, ii = np.meshgrid(np.arange(256), np.arange(256), indexing="ij")
    maskt = np.where(jj > ii, np.float32(NEG), np.float32(0.0))

    shared = {
        "wqk": w_qk.astype(BF),
        "whvg": np.concatenate([w_hidden[:, :HID], w_gate], axis=1).astype(BF),
        "wga": w_hidden[:, HID:].astype(BF),
        "wcomb": wcomb.astype(BF),
        "woa": w_out[HID:, :].astype(BF),
        "g46": np.concatenate(
            [(np.asarray(g4, np.float32) / CHUNK).T,
             (np.asarray(g2, np.float32) / CHUNK).T], axis=1).copy(),
        "b46": np.concatenate(
            [np.asarray(b4, np.float32).T,
             np.asarray(b2, np.float32).T], axis=1).copy(),
        "maskt": maskt,
        "identb": np.eye(128, dtype=np.float32).astype(BF),
    }
    if apply_g:
        shared["lng"] = np.broadcast_to(ln_g, (128, DIM)).copy()
    if apply_b:
        shared["lnb"] = np.broadcast_to(ln_b, (128, DIM)).copy()

    n_half = x.shape[1] // n_tok  # halves per batch row
    per_core = []
    for core in range(x.shape[0] * n_half):
        b = core // n_half
        h = core % n_half
        t0 = h * n_tok
        xp = np.zeros((n_tok + 128, DIM), np.float32)
        xp[128:] = x[b, t0:t0 + n_tok]
        if t0 > 0:
            xp[127] = x[b, t0 - 1]
        per_core.append({"xp": xp})
    return shared, per_core, apply_g, apply_b


def build_bass(n_tok, apply_g, apply_b, silu_native=True):
    nc = bacc.Bacc("TRN2", target_bir_lowering=False, debug=False,
                   num_devices=1)
    specs = {
        "xp": ([n_tok + 128, DIM], F32),
        "wqk": ([512, 128], BF16),
        "whvg": ([512, 1024], BF16),
        "wga": ([512, 512], BF16),
        "wcomb": ([512, 512], BF16),
        "woa": ([2048, 512], BF16),
        "g46": ([128, 6], F32),
        "b46": ([128, 6], F32),
        "maskt": ([256, 256], F32),
        "identb": ([128, 128], BF16),
    }
    if apply_g:
        specs["lng"] = ([128, 512], F32)
    if apply_b:
        specs["lnb"] = ([128, 512], F32)
    aps = {}
    for name, (shape, dt) in specs.items():
        aps[name] = nc.dram_tensor(name, shape, dt, kind="ExternalInput").ap()
    aps["y"] = nc.dram_tensor("y", [n_tok, DIM], F32,
                              kind="ExternalOutput").ap()

    with tile.TileContext(nc) as tc:
        with ExitStack() as ctx:
            build_core_program(ctx, tc, aps, n_tok, apply_g, apply_b,
                               silu_native=silu_native)
    nc.compile()
    return nc


def _run(inputs, trace=False, **spmd_kwargs):
    from concourse.bass_utils import run_bass_kernel_spmd

    shared, per_core, apply_g, apply_b = make_host_inputs(
        inputs["x"], inputs["ln_g"], inputs["ln_b"], inputs["w_qk"],
        inputs["g4"], inputs["b4"], inputs["g2"], inputs["b2"],
        inputs["w_hidden"], inputs["w_gate"], inputs["w_out"])

    nc = build_bass(T_CORE, apply_g, apply_b)

    in_maps = [{**shared, **pc} for pc in per_core]
    res = run_bass_kernel_spmd(nc, in_maps, core_ids=list(range(N_CORES)),
                               trace=trace, **spmd_kwargs)

    y = np.empty((B, SEQ, DIM), np.float32)
    n_half = SEQ // T_CORE
    for core in range(N_CORES):
        b = core // n_half
        h = core % n_half
        y[b, h * T_CORE:(h + 1) * T_CORE] = res.results[core]["y"]
    return y, res


def kernel(**inputs):
    return _run(inputs)[0]



# revision 32
# speedup vs baseline: 1.3967x; 1.3967x over previous
"""Trainium2 Bass kernel for nn_FLASH_ShareA_FFConvM.

Strategy: data-parallel over (batch, seq-half): 8 cores, each handling 4096
tokens (16 local-attention chunks of 256). All weights replicated. Per core:

  phase A (token-major, in blocks of ~11 tiles): LayerNorm stats with ONE
    batched sqrt per block (avoids ACT table thrash), normalized bf16 x
    stored token-major (xs_tok, reused for the residual add) and
    DMA-transposed into channel-major xsT [512c x 4224t] (col j = token
    j-128; 128-col halo at the front provides the token-shift source).
  phase B (per chunk pair): qk^T = wqk^T @ xs_sh^T; per-chunk mean ->
    OffsetScale with the softmax scale FOLDED into the q-side scales/offsets
    (host-side); q/k/qs/ks via DVE tensor_scalar (shift = col AP);
    sim^T[j,i] per head via PE with the causal mask ADDED VIA PE (identity
    matmul of a mask constant into the same PSUM accumulation);
    exp straight off PSUM in two wide [128,1024] ACT ops; softmax
    denominator via ones[128,128]-matmul (broadcasts the row-sum to all
    partitions, killing the DRAM-bounce broadcast); attn = exp * recip;
    V-matmul out^T[e,(h,i)]; silu -> og = silu * gate; fin[t,d] PSUM
    accumulates xs_sh@W_comb (folded hidden@w_out[:512]) + og@w_out[512:];
    finalize y = fin*silu(vgate) + xs_sh with the bf16 xs_tok copy
    (token-shift via small SBUF DMA).
  ACT ordering: exps for both chunks of a pair, then all silus -> 2 table
    loads per pair instead of ~5.
"""

import sys

sys.path.insert(0, "/opt/trn_rl_repo")

import numpy as np
import ml_dtypes
from contextlib import ExitStack

import concourse.bass as bass
import concourse.tile as tile
from concourse import bacc, mybir

F32 = mybir.dt.float32
BF16 = mybir.dt.bfloat16
AX = mybir.AxisListType
ALU = mybir.AluOpType
ACTF = mybir.ActivationFunctionType

B, SEQ, DIM = 4, 8192, 512
G, QK = 32, 128
CHUNK = SEQ // G          # 256 tokens per attention chunk
HD = QK // 4              # 32 (softmax scale dim, per source)
SCALE = float(HD) ** -0.5
HID = DIM                 # 512
EPS = 1e-5
N_CORES = 8
T_CORE = SEQ // 2         # 4096 tokens per core
NEG = -1.0e30

BF = ml_dtypes.bfloat16

SILU_NATIVE = True  # False: Sigmoid+mul fallback (CoreSim lacks Silu)
ONES_BC = True      # False: M=1 denominator + DRAM-bounce broadcast
STAGE = 100         # debug: truncate pipeline (100 = full; 71/72 = fin substages)


def build_core_program(ctx: ExitStack, tc, aps, n_tok, apply_g, apply_b):
    """Emit the per-core program. aps: dict name -> bass.AP (DRAM)."""
    nc = tc.nc
    n_tiles = n_tok // 128            # 128-token tiles (excl. halo tile)
    n_chunks = n_tok // CHUNK
    nt_all = n_tiles + 1              # + halo tile (rows 0..127 = token j-128)

    xp = aps["xp"]; yout = aps["y"]

    # ---------------- pools ----------------
    consts = ctx.enter_context(tc.tile_pool(name="consts", bufs=1))
    persist = ctx.enter_context(tc.tile_pool(name="persist", bufs=1))
    work = ctx.enter_context(tc.tile_pool(name="work", bufs=1))
    psum = ctx.enter_context(tc.tile_pool(name="psum", bufs=1, space="PSUM"))
    if not ONES_BC:
        dram = ctx.enter_context(tc.tile_pool(name="dram", bufs=1,
                                              space="DRAM"))

    # ---------------- constants into SBUF ----------------
    def cload(name, shape, dtype):
        """Load a [R, C] DRAM const into SBUF; R>128 folds to [128, R//128, C]
        (row r = cc*128 + p -> tile[p, cc, :]), returning slices via [:, cc, :].
        """
        if shape[0] > 128:
            k = shape[0] // 128
            t = consts.tile([128, k, shape[1]], dtype, name=f"c_{name}",
                            tag=f"c_{name}")
            nc.sync.dma_start(t[:], aps[name].rearrange("(k p) c -> p k c",
                                                        p=128))
        else:
            t = consts.tile(shape, dtype, name=f"c_{name}", tag=f"c_{name}")
            nc.sync.dma_start(t[:], aps[name])
        return t

    wqk = cload("wqk", [512, 128], BF16)      # [128c, 4cc, 128d]
    whvg = cload("whvg", [512, 1024], BF16)   # moving: cols 0:512 hid, 512: vgate
    wga = cload("wga", [512, 512], BF16)      # attn gate; lhsT tiles
    wcomb = cload("wcomb", [512, 512], BF16)  # moving rhs for fin part1
    woa = cload("woa", [2048, 512], BF16)     # moving rhs for fin part2
    g8 = cload("g8", [128, 7], F32)           # gammas.T (scaled, see host)
    b8 = cload("b8", [128, 7], F32)           # betas.T (scaled)
    bmask = cload("bmask", [128, 2048], BF16)  # [p,(jt,4h,256i)] 0/1 causal
    if apply_g:
        lng = cload("lng", [128, 512], F32)
    if apply_b:
        lnb = cload("lnb", [128, 512], F32)

    ones128 = consts.tile([128, 128 if ONES_BC else 1], BF16,
                          name="ones128", tag="ones128")
    nc.vector.memset(ones128[:], 1.0)
    epsb = consts.tile([128, 1], F32, name="epsb", tag="epsb")
    nc.vector.memset(epsb[:], EPS)

    # ---------------- persistent state ----------------
    # channel-major normalized x, bf16; col j = token (j - 128)
    # folded: xsT[p, cc, j] = xs[token j-128, channel cc*128+p]
    xsT = persist.tile([128, 4, n_tok + 128], BF16, name="xsT", tag="xsT")
    # token-major normalized x, bf16; slot i = tokens [(i-1)*128, i*128)
    xs_tok = persist.tile([128, nt_all, 512], BF16, name="xs_tok",
                          tag="xs_tok")
    # per-LN-tile stats: [:, 0, i] = mean, [:, 1, i] = rstd
    stats = persist.tile([128, 2, nt_all], F32, name="stats", tag="stats")

    def act(bi):
        """Chain ACT-table ops into a fixed order to avoid table thrash."""
        tc.chain_iter_dep("actfn", bi.ins)
        return bi

    # ---------------- phase A: LN in blocks (batched sqrt) ----------------
    x_keep = {}

    def emit_stats_block(i0, i1):
        for i in range(i0, i1):
            x_t = work.tile([128, 512], F32, name=f"xa{i}", tag="xa", bufs=12)
            nc.sync.dma_start(x_t[:], xp[i * 128:(i + 1) * 128, :])
            x_keep[i] = x_t
            bns = work.tile([128, 6], F32, name=f"bns{i}", tag="bns", bufs=2)
            nc.vector.bn_stats(out=bns[:], in_=x_t[:])
            mv = work.tile([128, 2], F32, name=f"mv{i}", tag="mv", bufs=2)
            nc.vector.bn_aggr(out=mv[:], in_=bns[:])
            nc.vector.tensor_copy(out=stats[:, 0, i:i + 1], in_=mv[:, 0:1])
            nc.vector.tensor_copy(out=stats[:, 1, i:i + 1], in_=mv[:, 1:2])
        # batched rstd = 1/sqrt(var+eps) over the block's contiguous var row
        vs = stats[:, 1, i0:i1]
        act(nc.scalar.activation(out=vs, in_=vs, func=ACTF.Sqrt,
                                 bias=epsb[:]))
        nc.vector.reciprocal(out=vs, in_=vs)

    def emit_ln_tile(i):
        x_t = x_keep.pop(i)
        mean = stats[:, 0, i:i + 1]
        rstd = stats[:, 1, i:i + 1]
        dst = xs_tok[:, i, :]
        if apply_g or apply_b:
            xs_f = work.tile([128, 512], F32, name=f"xsf{i}", tag="xsf",
                             bufs=2)
            nc.vector.tensor_scalar(out=xs_f[:], in0=x_t[:],
                                    scalar1=mean, scalar2=rstd,
                                    op0=ALU.subtract, op1=ALU.mult)
            if apply_g and apply_b:
                nc.vector.tensor_mul(out=xs_f[:], in0=xs_f[:], in1=lng[:])
                nc.vector.tensor_add(out=dst, in0=xs_f[:], in1=lnb[:])
            elif apply_g:
                nc.vector.tensor_mul(out=dst, in0=xs_f[:], in1=lng[:])
            else:
                nc.vector.tensor_add(out=dst, in0=xs_f[:], in1=lnb[:])
        else:
            nc.vector.tensor_scalar(out=dst, in0=x_t[:],
                                    scalar1=mean, scalar2=rstd,
                                    op0=ALU.subtract, op1=ALU.mult)
        # transpose [128t, 512c] -> xsT[:, :, i*128:(i+1)*128] in one DMA
        nc.sync.dma_start(xsT[:, :, i * 128:(i + 1) * 128], dst,
                          transpose=True)

    # stats blocks scheduled before pairs 0 / 2 / 4 (see below)
    n_pairs = n_chunks // 2
    sched_pairs = [pg for pg in (0, 2, 4) if pg < n_pairs] or [0]
    bsz = -(-nt_all // len(sched_pairs))
    bsz = max(bsz, 11)  # block k must cover ln tiles through 4*pg+6
    stats_sched = {}
    start = 0
    for pg in sched_pairs:
        if start >= nt_all:
            break
        end = nt_all if pg == sched_pairs[-1] else min(start + bsz, nt_all)
        stats_sched[pg] = (start, end)
        start = end

    ln_next = 0

    def ensure_ln(upto):
        nonlocal ln_next
        while ln_next <= min(upto, n_tiles):
            emit_ln_tile(ln_next)
            ln_next += 1

    def xsh(cc, col0, width):
        c0 = col0 - 1 if cc < 2 else col0
        return xsT[:, cc, c0:c0 + width]

    # ---------------- phase B: chunk pairs ----------------
    for gp in range(n_chunks // 2):
        if gp in stats_sched:
            emit_stats_block(*stats_sched[gp])
        ensure_ln(4 * gp + 6)
        colP = 128 + gp * 2 * CHUNK      # pair start col (unshifted)
        if STAGE < 2:
            continue

        # --- A) qk^T for the pair [128qk, 512] + gate^T (channel-major) ---
        qkps = psum.tile([128, 512], F32, name=f"qkps{gp}", tag="mm",
                         bufs=6)
        for cc in range(4):
            nc.tensor.matmul(qkps[:], wqk[:, cc, :], xsh(cc, colP, 512),
                             start=(cc == 0), stop=(cc == 3))
        qkT = work.tile([128, 512], BF16, name=f"qkT{gp}", tag="qkT", bufs=2)
        act(nc.scalar.copy(out=qkT[:], in_=qkps[:]))

        gate_bf = []
        for ee in range(4):
            gps = psum.tile([128, 512], F32, name=f"g{gp}_{ee}", tag="mm",
                            bufs=6)
            for cc in range(4):
                nc.tensor.matmul(gps[:], wga[:, cc, ee * 128:(ee + 1) * 128],
                                 xsh(cc, colP, 512),
                                 start=(cc == 0), stop=(cc == 3))
            gb = work.tile([128, 512], BF16, name=f"gate{gp}_{ee}",
                           tag=f"gate{ee}", bufs=2)
            act(nc.scalar.copy(out=gb[:], in_=gps[:]))
            gate_bf.append(gb)

        # --- B/C) fronts: offsets, q/k/qs/ks, sim + mask + exp ---
        expt_g = []
        for g in ((2 * gp, 2 * gp + 1) if STAGE >= 3 else ()):
            half = g % 2
            qk_c = qkT[:, half * 256:(half + 1) * 256]

            qsum = work.tile([128, 1], F32, name=f"qsum{g}", tag="qsum",
                             bufs=2)
            nc.vector.tensor_reduce(out=qsum[:], in_=qk_c, axis=AX.X,
                                    op=ALU.add)
            offs = work.tile([128, 7], F32, name=f"offs{g}", tag="offs",
                             bufs=2)
            # offs[:, i] = qk_sum * gamma_i/CHUNK + beta_i  (one DVE op)
            # cols: 0 qsc*S, 1 qoff*S, 2 qsc, 3 qsoff*S, 4 ksc, 5 koff, 6 ksoff
            nc.vector.scalar_tensor_tensor(out=offs[:], in0=g8[:],
                                           scalar=qsum[:], in1=b8[:],
                                           op0=ALU.mult, op1=ALU.add)

            qT = work.tile([128, 256], BF16, name=f"qT{g}", tag="qT", bufs=2)
            kT = work.tile([128, 256], BF16, name=f"kT{g}", tag="kT", bufs=2)
            nc.vector.tensor_scalar(out=qT[:], in0=qk_c,
                                    scalar1=offs[:, 0:1], scalar2=offs[:, 1:2],
                                    op0=ALU.mult, op1=ALU.add)
            nc.vector.tensor_scalar(out=kT[:], in0=qk_c,
                                    scalar1=offs[:, 4:5], scalar2=offs[:, 5:6],
                                    op0=ALU.mult, op1=ALU.add)
            qsT = work.tile([128, 256], BF16, name=f"qsT{g}", tag="qsT",
                            bufs=2)
            ksT = work.tile([128, 256], BF16, name=f"ksT{g}", tag="ksT",
                            bufs=2)
            nc.vector.tensor_copy(out=qsT[:, 0:1], in_=offs[:, 3:4])
            nc.vector.tensor_copy(out=ksT[:, 0:1], in_=offs[:, 6:7])
            nc.vector.tensor_scalar(out=qsT[:, 1:256], in0=qT[:, 0:255],
                                    scalar1=offs[:, 2:3], scalar2=offs[:, 3:4],
                                    op0=ALU.mult, op1=ALU.add)
            nc.vector.tensor_scalar(out=ksT[:, 1:256], in0=kT[:, 0:255],
                                    scalar1=offs[:, 4:5], scalar2=offs[:, 6:7],
                                    op0=ALU.mult, op1=ALU.add)

            if STAGE < 4:
                continue
            # sim^T per j-tile: [128j, 4h*256i]; mask added via PE (or DVE)
            expt = [work.tile([128, 1024], BF16, name=f"exp{g}_{jt}",
                              tag=f"exp{jt}", bufs=2) for jt in range(2)]
            for jt in range(2):
                for h in range(4):
                    Q = qT if h < 2 else qsT
                    K = kT if h < 2 else ksT
                    dr = (h % 2) * 64
                    simx = psum.tile([128, 256], F32,
                                     name=f"sim{g}_{jt}_{h}", tag="mm",
                                     bufs=6)
                    nc.tensor.matmul(
                        simx[:], K[dr:dr + 64, jt * 128:(jt + 1) * 128],
                        Q[dr:dr + 64, :], start=True, stop=True)
                    act(nc.scalar.activation(
                        out=expt[jt][:, h * 256:(h + 1) * 256],
                        in_=simx[:], func=ACTF.Exp))
                # causal mask as 0/1 multiply (logits are small: no overflow)
                nc.vector.tensor_mul(
                    out=expt[jt][:], in0=expt[jt][:],
                    in1=bmask[:, jt * 1024:(jt + 1) * 1024])
            expt_g.append(expt)

        # --- D/E) mids: denominators + attn; hidden/vgate matmuls ---
        attn_g = []
        hv_ps = []
        for gi, g in enumerate((2 * gp, 2 * gp + 1) if STAGE >= 5 else ()):
            expt = expt_g[gi]
            recb = work.tile([128, 1024], F32, name=f"recb{g}", tag="recb",
                             bufs=2)
            if ONES_BC:
                for s in range(2):
                    sums = psum.tile([128, 512], F32, name=f"sums{g}_{s}",
                                     tag="acc", bufs=2)
                    for jt in range(2):
                        nc.tensor.matmul(sums[:], ones128[:],
                                         expt[jt][:, s * 512:(s + 1) * 512],
                                         start=(jt == 0), stop=(jt == 1))
                    nc.vector.reciprocal_approx_fast(
                        out=recb[:, s * 512:(s + 1) * 512], in_=sums[:])
            else:
                rec = work.tile([1, 1024], F32, name=f"rec{g}", tag="rec",
                                bufs=2)
                for s in range(2):
                    sums = psum.tile([1, 512], F32, name=f"sums{g}_{s}",
                                     tag="acc", bufs=2)
                    for jt in range(2):
                        nc.tensor.matmul(sums[0:1, :], ones128[:],
                                         expt[jt][:, s * 512:(s + 1) * 512],
                                         start=(jt == 0), stop=(jt == 1))
                    nc.vector.reciprocal_approx_fast(
                        out=rec[:, s * 512:(s + 1) * 512], in_=sums[0:1, :])
                recd = dram.tile([1, 1024], F32, name=f"recd{g}", tag="recd",
                                 bufs=2)
                nc.sync.dma_start(recd[:], rec[:])
                bcast = bass.AP(tensor=recd.tensor, offset=recd.offset,
                                ap=[[0, 128], list(recd.ap)[-1]])
                nc.sync.dma_start(recb[:], bcast)
            attn = [work.tile([128, 1024], BF16, name=f"attn{g}_{jt}",
                              tag=f"attn{jt}", bufs=2) for jt in range(2)]
            for jt in range(2):
                nc.vector.tensor_mul(out=attn[jt][:], in0=expt[jt][:],
                                     in1=recb[:])
            attn_g.append(attn)

            # hidden + vgate matmuls for the chunk's 2 t-tiles (PE filler);
            # cols 0:512 hidden, 512:1024 vgate in one 2-bank tile
            colC = 128 + g * CHUNK
            if STAGE < 6:
                hv_ps.append(None)
                continue
            ps = []
            for tt in range(2):
                colT = colC + tt * 128
                halves = [psum.tile([128, 512], F32,
                                    name=f"hv{g}_{tt}_{s}", tag="mm",
                                    bufs=6)[:] for s in range(2)]
                for s in range(2):
                    for cc in range(4):
                        nc.tensor.matmul(halves[s],
                                         xsh(cc, colT, 128),
                                         whvg[:, cc, s * 512:(s + 1) * 512],
                                         start=(cc == 0), stop=(cc == 3))
                ps.append(halves)
            hv_ps.append(ps)

        # hid copies + vgate silus for BOTH chunks (frees the hv PSUM slots
        # before the V matmuls rotate into them)
        hid_g = []
        svg_g = []
        for gi, g in enumerate((2 * gp, 2 * gp + 1) if STAGE >= 6 else ()):
            hid_bf = []
            svg_bf = []
            for tt in range(2):
                hvh, hvv2 = hv_ps[gi][tt]
                hb = work.tile([128, 512], BF16, name=f"hid{g}_{tt}",
                               tag="hid", bufs=4)
                act(nc.scalar.copy(out=hb[:], in_=hvh))
                sv = work.tile([128, 512], BF16, name=f"svg{g}_{tt}",
                               tag="svg", bufs=4)
                if SILU_NATIVE:
                    act(nc.scalar.activation(out=sv[:], in_=hvv2,
                                             func=ACTF.Silu))
                else:
                    sgt = work.tile([128, 512], F32, name=f"sgt{g}_{tt}",
                                    tag="sgt", bufs=2)
                    act(nc.scalar.activation(out=sgt[:], in_=hvv2,
                                             func=ACTF.Sigmoid))
                    nc.vector.tensor_mul(out=sv[:], in0=sgt[:],
                                         in1=hvv2)
                hid_bf.append(hb)
                svg_bf.append(sv)
            hid_g.append(hid_bf)
            svg_g.append(svg_bf)

        # --- F/G) backs: V, og, fin, finalize ---
        for gi, g in enumerate((2 * gp, 2 * gp + 1) if STAGE >= 7 else ()):
            half = g % 2
            attn = attn_g[gi]
            hid_bf = hid_g[gi]
            svg_bf = svg_g[gi]

            og_bf = []
            for ee in range(4):
                vsubs = [psum.tile([128, 512], F32,
                                   name=f"v{g}_{ee}_{s}", tag="mm",
                                   bufs=6)[:] for s in range(2)]
                for s in range(2):
                    for jt in range(2):
                        nc.tensor.matmul(
                            vsubs[s],
                            hid_bf[jt][:, ee * 128:(ee + 1) * 128],
                            attn[jt][:, s * 512:(s + 1) * 512],
                            start=(jt == 0), stop=(jt == 1))
                osl = work.tile([128, 1024], BF16, name=f"osl{g}_{ee}",
                                tag="osl", bufs=2)
                for s in range(2):
                    sl = slice(s * 512, (s + 1) * 512)
                    if SILU_NATIVE:
                        act(nc.scalar.activation(out=osl[:, sl],
                                                 in_=vsubs[s],
                                                 func=ACTF.Silu))
                    else:
                        sgo = work.tile([128, 512], F32,
                                        name=f"sgo{g}_{ee}_{s}",
                                        tag="sgo", bufs=2)
                        act(nc.scalar.activation(out=sgo[:], in_=vsubs[s],
                                                 func=ACTF.Sigmoid))
                        nc.vector.tensor_mul(out=osl[:, sl], in0=sgo[:],
                                             in1=vsubs[s])
                ob = work.tile([128, 1024], BF16, name=f"og{g}_{ee}",
                               tag=f"og{ee}", bufs=2)
                gslice = gate_bf[ee][:, half * 256:(half + 1) * 256]
                gbc = gslice.unsqueeze(1).broadcast_to((128, 4, 256))
                nc.vector.tensor_mul(
                    out=ob.rearrange("p (h i) -> p h i", h=4),
                    in0=osl.rearrange("p (h i) -> p h i", h=4),
                    in1=gbc)
                og_bf.append(ob)

            for tt in range(2 if STAGE >= 71 else 0):
                ti = g * 2 + tt
                colT = 128 + ti * 128
                fin = psum.tile([128, 512], F32, name=f"fin{g}_{tt}",
                                tag="acc", bufs=2)
                for cc in range(4):
                    nc.tensor.matmul(fin[:], xsh(cc, colT, 128),
                                     wcomb[:, cc, :],
                                     start=(cc == 0), stop=False)
                for h in range(4):
                    for ee in range(4):
                        ff = h * 4 + ee
                        nc.tensor.matmul(
                            fin[:],
                            og_bf[ee][:, h * 256 + tt * 128:
                                      h * 256 + tt * 128 + 128],
                            woa[:, ff, :],
                            start=False, stop=(ff == 15))

                if STAGE < 72:
                    continue
                y = work.tile([128, 512], F32, name=f"y{ti}", tag="y",
                              bufs=3)
                nc.vector.tensor_mul(out=y[:], in0=fin[:],
                                     in1=svg_bf[tt][:])
                if STAGE >= 100:
                    xsprev = work.tile([128, 256], BF16, name=f"xsp{ti}",
                                       tag="xsp", bufs=2)
                    nc.sync.dma_start(xsprev[1:128, :],
                                      xs_tok[0:127, ti + 1, 0:256])
                    nc.sync.dma_start(xsprev[0:1, :],
                                      xs_tok[127:128, ti, 0:256])
                    nc.vector.tensor_add(out=y[:, 256:512],
                                         in0=y[:, 256:512],
                                         in1=xs_tok[:, ti + 1, 256:512])
                    nc.vector.tensor_add(out=y[:, 0:256], in0=y[:, 0:256],
                                         in1=xsprev[:])
                nc.sync.dma_start(yout[ti * 128:(ti + 1) * 128, :], y[:])


    if STAGE < 72:
        ensure_ln(n_tiles)
        for ti in range(n_tiles):
            yc = work.tile([128, 512], F32, name=f"dbg{ti}", tag="y", bufs=3)
            nc.vector.tensor_copy(out=yc[:], in_=xs_tok[:, ti + 1, :])
            nc.sync.dma_start(yout[ti * 128:(ti + 1) * 128, :], yc[:])


def make_host_inputs(x, ln_g, ln_b, w_qk, g4, b4, g2, b2, w_hidden, w_gate,
                     w_out, n_tok=T_CORE):
    """Build shared weight arrays + per-core xp slices."""
    x = np.asarray(x, np.float32)
    ln_g = np.asarray(ln_g, np.float32)
    ln_b = np.asarray(ln_b, np.float32)
    apply_g = not np.all(ln_g == 1.0)
    apply_b = bool(np.any(ln_b != 0.0))

    w_hidden = np.asarray(w_hidden, np.float32)
    w_out = np.asarray(w_out, np.float32)
    w_gate = np.asarray(w_gate, np.float32)
    w_qk = np.asarray(w_qk, np.float32)

    wcomb = (w_hidden[:, :HID] @ w_out[:HID, :]).astype(np.float32)

    # offset/scale gammas+betas with softmax scale folded into the q side.
    # cols: 0 qsc*S, 1 qoff*S, 2 qsc, 3 qsoff*S, 4 ksc, 5 koff, 6 ksoff
    g4n = np.asarray(g4, np.float32) / CHUNK     # [4, 128] rows q_off,k_off,q_sc,k_sc
    b4n = np.asarray(b4, np.float32)
    g2n = np.asarray(g2, np.float32) / CHUNK     # rows q_s_off, k_s_off
    b2n = np.asarray(b2, np.float32)
    S = SCALE
    g8 = np.stack([g4n[2] * S, g4n[0] * S, g4n[2], g2n[0] * S,
                   g4n[3], g4n[1], g2n[1]], axis=1).copy()   # [128, 7]
    b8 = np.stack([b4n[2] * S, b4n[0] * S, b4n[2], b2n[0] * S,
                   b4n[3], b4n[1], b2n[1]], axis=1).copy()

    # 0/1 causal mask, keys-major: bmask[p, (jt, h, i)] = jt*128+p <= i
    jj, ii = np.meshgrid(np.arange(256), np.arange(256), indexing="ij")
    maskt = np.where(jj > ii, np.float32(0.0), np.float32(1.0))  # [256j,256i]
    m4 = maskt.reshape(2, 128, 256).transpose(1, 0, 2)           # [128p,2jt,256i]
    bmask = np.concatenate([m4] * 4, axis=2).reshape(128, 2048)  # dup over 4h

    shared = {
        "wqk": w_qk.astype(BF),
        "whvg": np.concatenate([w_hidden[:, :HID], w_gate], axis=1).astype(BF),
        "wga": w_hidden[:, HID:].astype(BF),
        "wcomb": wcomb.astype(BF),
        "woa": w_out[HID:, :].astype(BF),
        "g8": g8,
        "b8": b8,
        "bmask": bmask.astype(BF),
    }
    if apply_g:
        shared["lng"] = np.broadcast_to(ln_g, (128, DIM)).copy()
    if apply_b:
        shared["lnb"] = np.broadcast_to(ln_b, (128, DIM)).copy()

    n_half = x.shape[1] // n_tok  # halves per batch row
    per_core = []
    for core in range(x.shape[0] * n_half):
        b = core // n_half
        h = core % n_half
        t0 = h * n_tok
        xp = np.zeros((n_tok + 128, DIM), np.float32)
        xp[128:] = x[b, t0:t0 + n_tok]
        if t0 > 0:
            xp[127] = x[b, t0 - 1]
        per_core.append({"xp": xp})
    return shared, per_core, apply_g, apply_b


def build_bass(n_tok, apply_g, apply_b):
    nc = bacc.Bacc("TRN2", target_bir_lowering=False, debug=False,
                   num_devices=1)
    specs = {
        "xp": ([n_tok + 128, DIM], F32),
        "wqk": ([512, 128], BF16),
        "whvg": ([512, 1024], BF16),
        "wga": ([512, 512], BF16),
        "wcomb": ([512, 512], BF16),
        "woa": ([2048, 512], BF16),
        "g8": ([128, 7], F32),
        "b8": ([128, 7], F32),
        "bmask": ([128, 2048], BF16),
    }
    if apply_g:
        specs["lng"] = ([128, 512], F32)
    if apply_b:
        specs["lnb"] = ([128, 512], F32)
    aps = {}
    for name, (shape, dt) in specs.items():
        aps[name] = nc.dram_tensor(name, shape, dt, kind="ExternalInput").ap()
    aps["y"] = nc.dram_tensor("y", [n_tok, DIM], F32,
                              kind="ExternalOutput").ap()

    with tile.TileContext(nc) as tc:
        with ExitStack() as ctx:
            build_core_program(ctx, tc, aps, n_tok, apply_g, apply_b)
    nc.compile()
    return nc


def _run(inputs, trace=False, **spmd_kwargs):
    from concourse.bass_utils import run_bass_kernel_spmd

    shared, per_core, apply_g, apply_b = make_host_inputs(
        inputs["x"], inputs["ln_g"], inputs["ln_b"], inputs["w_qk"],
        inputs["g4"], inputs["b4"], inputs["g2"], inputs["b2"],
        inputs["w_hidden"], inputs["w_gate"], inputs["w_out"])

    nc = build_bass(T_CORE, apply_g, apply_b)

    in_maps = [{**shared, **pc} for pc in per_core]
    res = run_bass_kernel_spmd(nc, in_maps, core_ids=list(range(N_CORES)),
                               trace=trace, **spmd_kwargs)

    y = np.empty((B, SEQ, DIM), np.float32)
    n_half = SEQ // T_CORE
    for core in range(N_CORES):
        b = core // n_half
        h = core % n_half
        y[b, h * T_CORE:(h + 1) * T_CORE] = res.results[core]["y"]
    return y, res


def kernel(**inputs):
    return _run(inputs)[0]
